# revision 10
# baseline (speedup 1.0000x reference)
"""BiLSTM parser kernel for Trainium2 (Bass/Tile), 8-core SPMD.

Time-sharded fixed-point solver. Core k (c = k//2, dir = k%2) processes the
global window [256c - W, 256(c+1) + W) of the sequence in direction dir
(dir 1 = backward; its inputs are time-flipped on the host so all cores run
an identical program). The LSTM recurrence is solved by ns fixed-point
sweeps per layer (gates = pre + Whh @ h_prev batched over the window; c
solved exactly by tensor_tensor_scan; h = o*tanh(c)). Window boundary error
decays ~0.55^j into the window, so W=32 warm-up steps make the per-pair
valid block [256c, 256(c+1)) accurate with ZERO communication during
sweeps. Sentinel padding (pre += -40) outside t in [0, 1024) forces h=c=0
exactly, so edge cores run the same program.

Between layers, the fwd/bwd partner pair exchanges its window h via a
2-party AllReduce (partner = sum - own). For the score, fwd cores place
their valid hs/ms block through a per-core one-hot placement matmul into a
[128, 16] tile, an 8-party AllReduce assembles the global vectors, and
every core redundantly computes the full [1024, 1024] score matrix.

Layouts (per direction, as in the 2-core version):
  - gates as [128 partitions, 16 chunks] = 128-row chunks in order
    m = typ*4 + hc, typ in [i, f, o, g], hc = 128-chunk of the 512 h dims
  - h stored as [128, t*4 + hc], the transposed layout the next matmul
    consumes as rhs.
"""

import numpy as np

F16 = np.float16

L = 1024
H = 512
G4 = 2048
DW, DP = 300, 100
IN0 = DW + DP      # 400
IN0P = 512
IN1 = 1024
MH = 512
NB = 4             # time blocks (core pairs)
B = L // NB        # 256

# gate chunk order [i, f, o, g] (ref order is [i, f, g, o])
P_ROWS = np.concatenate([
    np.arange(0, 512), np.arange(512, 1024),
    np.arange(1536, 2048), np.arange(1024, 1536),
])


# ---------------------------------------------------------------- host packing

def _pack_lhsT_image(w_perm: np.ndarray, kc: int) -> np.ndarray:
    """w_perm [M_out, K_in] -> SBUF image [128, kc*M_out],
    col = k*M_out + m*128 + q, img[p, ...] = w_perm[m*128+q, k*128+p]."""
    m_out, k_in = w_perm.shape
    assert k_in == kc * 128 and m_out % 128 == 0
    mc = m_out // 128
    img = w_perm.reshape(mc, 128, kc, 128).transpose(3, 2, 0, 1).reshape(128, kc * m_out)
    return np.ascontiguousarray(img.astype(F16))


def _pack_xT_image(x_pad: np.ndarray, kc: int) -> np.ndarray:
    """x_pad [T, kc*128] -> [128, kc*T], col = k*T + t."""
    t_len, k_in = x_pad.shape
    assert k_in == kc * 128
    img = x_pad.reshape(t_len, kc, 128).transpose(2, 1, 0).reshape(128, kc * t_len)
    return np.ascontiguousarray(img.astype(F16))


def _pack_bias16(b_perm: np.ndarray) -> np.ndarray:
    return np.ascontiguousarray(b_perm.reshape(16, 128).T.astype(np.float32))


def _pack_col4(v: np.ndarray) -> np.ndarray:
    return np.ascontiguousarray(v.reshape(4, 128).T)


def _dir_weights(gi: dict, d: int) -> dict:
    """Per-direction weight images (shared by the 4 cores of direction d)."""
    m = {}
    w0 = gi["Wih_l0"][d].astype(np.float32)[P_ROWS]
    w0p = np.zeros((G4, IN0P), np.float32)
    w0p[:, :IN0] = w0
    m["wih0"] = _pack_lhsT_image(w0p, IN0P // 128)
    m["whh0"] = _pack_lhsT_image(gi["Whh_l0"][d].astype(np.float32)[P_ROWS], H // 128)
    m["b0"] = _pack_bias16((gi["bih_l0"][d] + gi["bhh_l0"][d]).astype(np.float32)[P_ROWS])

    w1 = gi["Wih_l1"][d].astype(np.float32)[P_ROWS]
    if d == 1:  # own (bwd) features first
        w1 = np.concatenate([w1[:, H:], w1[:, :H]], axis=1)
    m["wih1"] = _pack_lhsT_image(w1, IN1 // 128)
    m["whh1"] = _pack_lhsT_image(gi["Whh_l1"][d].astype(np.float32)[P_ROWS], H // 128)
    m["b1"] = _pack_bias16((gi["bih_l1"][d] + gi["bhh_l1"][d]).astype(np.float32)[P_ROWS])

    wh = gi["W_head"].astype(np.float32)
    wm = gi["W_modif"].astype(np.float32)
    if d == 1:
        wh = np.concatenate([wh[:, H:], wh[:, :H]], axis=1)
        wm = np.concatenate([wm[:, H:], wm[:, :H]], axis=1)
    m["whead"] = _pack_lhsT_image(wh, IN1 // 128)
    m["wmodif"] = _pack_lhsT_image(wm, IN1 // 128)
    m["bhm"] = np.concatenate(
        [_pack_col4(gi["b_head"].astype(np.float32)),
         _pack_col4(gi["b_modif"].astype(np.float32))], axis=1)
    m["wout"] = np.ascontiguousarray(np.concatenate(
        [_pack_col4(gi["W_out"][0, :MH].astype(np.float32)),
         _pack_col4(gi["W_out"][0, MH:].astype(np.float32))], axis=1).astype(F16))
    return m


def pack_inputs(inputs: dict, w_warm: int = 32) -> list[dict]:
    """Full problem inputs -> 8 per-core in_maps."""
    gi = {k: np.asarray(v) for k, v in inputs.items()}
    wid = gi["word_tensor"].astype(np.int64).reshape(-1)[:L]
    pid = gi["pos_tensor"].astype(np.int64).reshape(-1)[:L]
    x = np.concatenate([gi["word_emb"].astype(np.float32)[wid],
                        gi["pos_emb"].astype(np.float32)[pid]], axis=1)  # [L, 400]

    wn = B + 2 * w_warm
    dir_w = [_dir_weights(gi, 0), _dir_weights(gi, 1)]

    in_maps = []
    for k in range(2 * NB):
        c, d = k // 2, k % 2
        lo = B * c - w_warm
        ts = lo + np.arange(wn)            # global t per local index (fwd order)
        if d == 1:
            ts = ts[::-1]
        valid = (ts >= 0) & (ts < L)
        xw = np.zeros((wn, IN0P), np.float32)
        xw[valid, :IN0] = x[np.clip(ts, 0, L - 1)][valid]
        m = dict(dir_w[d])
        m["xT"] = _pack_xT_image(xw, IN0P // 128)
        maskrow = np.where(valid, 0.0, -40.0).astype(np.float32)
        m["mask"] = np.ascontiguousarray(np.broadcast_to(maskrow, (128, wn)).copy())
        smat = np.zeros((4, 16), np.float32)
        if d == 0:
            smat[0, 2 * c] = 1.0       # hs chunk 2c
            smat[1, 2 * c + 1] = 1.0   # hs chunk 2c+1
            smat[2, 8 + 2 * c] = 1.0   # ms chunk 2c
            smat[3, 8 + 2 * c + 1] = 1.0
        m["smat"] = smat
        s2 = np.zeros((16, 1), np.float32)
        s2[8 + k, 0] = 1.0     # core k owns score rows [128k, 128k+128)
        m["s2"] = s2
        in_maps.append(m)
    return in_maps


# ---------------------------------------------------------------- device build

def build_nc(ns0: int = 10, ns1: int = 10, w_warm: int = 32, b_out: float = 0.0,
             one_core: bool = False, use_ident: bool = False, ps_bufs: int = 8,
             upto: str = "full"):
    import concourse.bacc as bacc
    import concourse.tile as tile
    from concourse import mybir
    from concourse.masks import make_identity

    f32 = mybir.dt.float32
    f16 = mybir.dt.float16
    AF = mybir.ActivationFunctionType

    wn = B + 2 * w_warm
    n_dev = 1 if one_core else 2 * NB
    pair_groups = [[2 * c, 2 * c + 1] for c in range(NB)]
    all_group = [list(range(2 * NB))]

    nc = bacc.Bacc("TRN2", target_bir_lowering=False, debug=False, num_devices=n_dev)

    xT = nc.dram_tensor("xT", [128, 4 * wn], f16, kind="ExternalInput")
    mask = nc.dram_tensor("mask", [128, wn], f32, kind="ExternalInput")
    wih0 = nc.dram_tensor("wih0", [128, 4 * G4], f16, kind="ExternalInput")
    whh0 = nc.dram_tensor("whh0", [128, 4 * G4], f16, kind="ExternalInput")
    b0 = nc.dram_tensor("b0", [128, 16], f32, kind="ExternalInput")
    wih1 = nc.dram_tensor("wih1", [128, 8 * G4], f16, kind="ExternalInput")
    whh1 = nc.dram_tensor("whh1", [128, 4 * G4], f16, kind="ExternalInput")
    b1 = nc.dram_tensor("b1", [128, 16], f32, kind="ExternalInput")
    whead = nc.dram_tensor("whead", [128, 8 * MH], f16, kind="ExternalInput")
    wmodif = nc.dram_tensor("wmodif", [128, 8 * MH], f16, kind="ExternalInput")
    bhm = nc.dram_tensor("bhm", [128, 8], f32, kind="ExternalInput")
    wout = nc.dram_tensor("wout", [128, 8], f16, kind="ExternalInput")
    smat = nc.dram_tensor("smat", [4, 16], f32, kind="ExternalInput")
    s2 = nc.dram_tensor("s2", [16, 1], f32, kind="ExternalInput")
    score = nc.dram_tensor("score", [128, L], f32, kind="ExternalOutput")

    ar0_in = nc.dram_tensor("ar0_in", [128, 4 * wn], f16)
    ar0_out = nc.dram_tensor("ar0_out", [128, 4 * wn], f16)
    ar1_in = nc.dram_tensor("ar1_in", [128, 4 * wn], f16)
    ar1_out = nc.dram_tensor("ar1_out", [128, 4 * wn], f16)
    hm_in = nc.dram_tensor("hm_in", [128, 16], f32)
    hm_out = nc.dram_tensor("hm_out", [128, 16], f32)

    _stages = ["p0", "l0", "x0", "p2", "l1", "x1", "p4f", "full"]
    _lim = _stages.index(upto)

    def go(st):
        return _stages.index(st) <= _lim

    with tile.TileContext(nc) as tc:
        with tc.tile_pool(name="persist", bufs=1) as pers:
            pre_sb = pers.tile([128, 16 * wn], f32)
            h0 = pers.tile([128, 4 * wn + 4], f16)
            h_tmp = pers.tile([128, 4 * wn + 4], f16)
            h1 = pers.tile([128, 4 * wn + 4], f16)
            h0p = pers.tile([128, 4 * wn], f16)
            h1p = pers.tile([128, 4 * wn], f16)
            whh_sb = pers.tile([128, 4 * G4], f16)
            wih1_sb = pers.tile([128, 8 * G4], f16)
            mask_sb = pers.tile([128, wn], f32)

            pre_v = pre_sb[:, :].rearrange("p (m t) -> p m t", m=16)

            nc.vector.memset(h0[:, 0:4], 0.0)
            nc.vector.memset(h_tmp[:, 0:4], 0.0)
            nc.vector.memset(h1[:, 0:4], 0.0)
            id32 = pers.tile([128, 128], f32)
            make_identity(nc, id32[:, :])
            ones128 = pers.tile([1, 128], f32)
            nc.vector.memset(ones128[:, :], 1.0)

            ident = None
            if use_ident:
                ident = pers.tile([128, 128], f16)
                make_identity(nc, ident[:, :])

            # ---------------- P0: pre = Wih0 @ x + b0 + mask
            with tc.tile_pool(name="p0", bufs=1) as p0pool, \
                 tc.tile_pool(name="p0ps", bufs=4, space="PSUM") as p0ps:
                wih0_sb = p0pool.tile([128, 4 * G4], f16)
                xT_sb = p0pool.tile([128, 4 * wn], f16)
                for k in range(4):
                    nc.sync.dma_start(out=xT_sb[:, k * wn:(k + 1) * wn],
                                      in_=xT[:, k * wn:(k + 1) * wn])
                    nc.sync.dma_start(out=wih0_sb[:, k * G4:(k + 1) * G4],
                                      in_=wih0[:, k * G4:(k + 1) * G4])
                b0_sb = p0pool.tile([128, 16], f32)
                nc.sync.dma_start(out=b0_sb[:, :], in_=b0[:, :])
                nc.sync.dma_start(out=mask_sb[:, :], in_=mask[:, :])
                for m in range(16):
                    ps = p0ps.tile([128, wn], f32)
                    for k in range(4):
                        nc.tensor.matmul(
                            ps[:, :],
                            wih0_sb[:, (k * 16 + m) * 128:(k * 16 + m + 1) * 128],
                            xT_sb[:, k * wn:(k + 1) * wn],
                            start=(k == 0), stop=(k == 3))
                    nc.vector.tensor_scalar_add(pre_v[:, m, :], ps[:, :], b0_sb[:, m:m + 1])
                    nc.vector.tensor_add(pre_v[:, m, :], pre_v[:, m, :], mask_sb[:, :])

            # prefetch recurrence + layer-1 weights
            nc.sync.dma_start(out=whh_sb[:, :], in_=whh0[:, :])
            nc.sync.dma_start(out=wih1_sb[:, :], in_=wih1[:, :])

            # ---------------- fixed-point sweeps (unrolled)
            def emit_sweeps(n_sweeps, buf_even, buf_odd, nm):
                with tc.tile_pool(name=f"sw_ps{nm}", bufs=ps_bufs, space="PSUM") as sps, \
                     tc.tile_pool(name=f"sw_sb{nm}", bufs=1) as ssb:
                    gI = ssb.tile([128, 4 * wn], f16)
                    gF = ssb.tile([128, 4 * wn], f16)
                    gO = ssb.tile([128, 4 * wn], f16)
                    gG = ssb.tile([128, 4 * wn], f16)
                    gC = ssb.tile([128, 4 * wn], f16)
                    gate_tiles = [gI, gF, gO, gG]
                    funcs = [AF.Sigmoid, AF.Sigmoid, AF.Sigmoid, AF.Tanh]

                    def cell_scan(hc):
                        cs = slice(hc * wn, (hc + 1) * wn)
                        nc.vector.tensor_mul(gI[:, cs], gI[:, cs], gG[:, cs])
                        nc.vector.tensor_tensor_scan(
                            gC[:, cs], gF[:, cs], gI[:, cs], 0.0,
                            mybir.AluOpType.mult, mybir.AluOpType.add)

                    def cell_out(dstv, hc):
                        cs = slice(hc * wn, (hc + 1) * wn)
                        nc.scalar.activation(gC[:, cs], gC[:, cs], AF.Tanh)
                        nc.vector.tensor_mul(dstv[:, :, hc], gO[:, cs], gC[:, cs])

                    for s in range(n_sweeps):
                        dst = buf_even if s % 2 == 0 else buf_odd
                        src = None if s == 0 else (buf_odd if s % 2 == 0 else buf_even)
                        dstv = dst[:, 4:4 + 4 * wn].rearrange("p (t k) -> p t k", k=4)
                        srcv = None if src is None else \
                            src[:, 0:4 * wn].rearrange("p (t k) -> p t k", k=4)
                        if src is None:
                            # gates = act(pre); pre is m-major so the 4 hc of one
                            # typ are contiguous: one [128, 4*wn] act per gate type
                            for typ in range(4):
                                nc.scalar.activation(
                                    gate_tiles[typ][:, :],
                                    pre_sb[:, (typ * 4) * wn:(typ * 4 + 4) * wn],
                                    funcs[typ])
                            for hc in range(4):
                                cell_scan(hc)
                                cell_out(dstv, hc)
                        else:
                            for hc in range(4):
                                for typ in range(4):
                                    m = typ * 4 + hc
                                    gslice = gate_tiles[typ][:, hc * wn:(hc + 1) * wn]
                                    ps = sps.tile([128, wn], f32, tag="g")
                                    nc.vector.tensor_copy(ps[:, :], pre_v[:, m, :])
                                    for k in range(4):
                                        nc.tensor.matmul(
                                            ps[:, :],
                                            whh_sb[:, (k * 16 + m) * 128:(k * 16 + m + 1) * 128],
                                            srcv[:, :, k],
                                            start=False, stop=(k == 3),
                                            skip_group_check=True)
                                    nc.scalar.activation(gslice, ps[:, :], funcs[typ])
                                cell_scan(hc)
                                cell_out(dstv, hc)
                    return buf_even if (n_sweeps - 1) % 2 == 0 else buf_odd

            # ---------------- L0 recurrence
            h0f = emit_sweeps(ns0, h0, h_tmp, 0) if go("l0") else None

            # ---------------- exchange: partner = pair-allreduce - own
            def emit_exchange(h_tile, hp_tile, ar_in, ar_out, nm):
                with tc.tile_pool(name=f"xch{nm}", bufs=1) as xp:
                    nc.sync.dma_start(out=ar_in[:, :], in_=h_tile[:, 4:4 + 4 * wn])
                    if one_core:
                        nc.sync.dma_start(out=ar_out[:, :], in_=ar_in[:, :])
                    else:
                        nc.gpsimd.collective_compute(
                            "AllReduce", mybir.AluOpType.add,
                            ins=[ar_in[:, :]], outs=[ar_out[:, :]],
                            replica_groups=pair_groups)
                    sum16 = xp.tile([128, 4 * wn], f16)
                    nc.sync.dma_start(out=sum16[:, :], in_=ar_out[:, :])
                    nc.vector.tensor_sub(hp_tile[:, :], sum16[:, :], h_tile[:, 4:4 + 4 * wn])

            if go("x0"):
                emit_exchange(h0f, h0p, ar0_in, ar0_out, 0)

            # ---------------- P2: pre = Wih1 @ [own | partner-flipped] + b1 + mask
            if go("p2"):
                h0v = h0f[:, 4:4 + 4 * wn].rearrange("p (t k) -> p t k", k=4)
                h0pv = h0p[:, :].rearrange("p (t k) -> p t k", k=4)[:, ::-1, :]
                with tc.tile_pool(name="p2ps", bufs=4, space="PSUM") as p2ps, \
                     tc.tile_pool(name="p2b", bufs=1) as p2bpool:
                    b1_sb = p2bpool.tile([128, 16], f32)
                    nc.sync.dma_start(out=b1_sb[:, :], in_=b1[:, :])
                    for m in range(16):
                        ps = p2ps.tile([128, wn], f32)
                        for k in range(8):
                            rhs = h0v[:, :, k] if k < 4 else h0pv[:, :, k - 4]
                            nc.tensor.matmul(
                                ps[:, :],
                                wih1_sb[:, (k * 16 + m) * 128:(k * 16 + m + 1) * 128],
                                rhs, start=(k == 0), stop=(k == 7))
                        nc.vector.tensor_scalar_add(pre_v[:, m, :], ps[:, :], b1_sb[:, m:m + 1])
                        nc.vector.tensor_add(pre_v[:, m, :], pre_v[:, m, :], mask_sb[:, :])

                nc.sync.dma_start(out=whh_sb[:, :], in_=whh1[:, :])

            # ---------------- L1 recurrence
            h1f = emit_sweeps(ns1, h1, h_tmp, 1) if go("l1") else None

            if go("x1"):
                emit_exchange(h1f, h1p, ar1_in, ar1_out, 1)

            # ---------------- P4: head/modif features + global score
            if not go("p4f"):
                nc2 = None
            h1v = h1f[:, 4:4 + 4 * wn].rearrange("p (t k) -> p t k", k=4)
            h1pv = h1p[:, :].rearrange("p (t k) -> p t k", k=4)[:, ::-1, :]
            with tc.tile_pool(name="p4", bufs=1) as p4pool, \
                 tc.tile_pool(name="p4ps", bufs=2, space="PSUM") as p4ps, \
                 tc.tile_pool(name="p4ps1", bufs=3, space="PSUM") as p4ps1, \
                 tc.tile_pool(name="p4sc", bufs=3) as p4sc:
                whead_sb = p4pool.tile([128, 8 * MH], f16)
                nc.sync.dma_start(out=whead_sb[:, :], in_=whead[:, :])
                wmodif_sb = p4pool.tile([128, 8 * MH], f16)
                nc.sync.dma_start(out=wmodif_sb[:, :], in_=wmodif[:, :])
                bhm_sb = p4pool.tile([128, 8], f32)
                nc.sync.dma_start(out=bhm_sb[:, :], in_=bhm[:, :])
                wout_sb = p4pool.tile([128, 8], f16)
                nc.sync.dma_start(out=wout_sb[:, :], in_=wout[:, :])
                smat_sb = p4pool.tile([4, 16], f32)
                nc.sync.dma_start(out=smat_sb[:, :], in_=smat[:, :])
                id32 = p4pool.tile([128, 128], f32)
                make_identity(nc, id32[:, :])

                th_sb = p4pool.tile([128, 4 * wn], f16)
                tm_sb = p4pool.tile([128, 4 * wn], f16)
                th_v = th_sb[:, :].rearrange("p (t m) -> p t m", m=4)
                tm_v = tm_sb[:, :].rearrange("p (t m) -> p t m", m=4)

                for (w_sb, out_v, bcol) in ((whead_sb, th_v, 0), (wmodif_sb, tm_v, 4)):
                    for m in range(4):
                        ps = p4ps.tile([128, wn], f32, tag="mlp")
                        for k in range(8):
                            rhs = h1v[:, :, k] if k < 4 else h1pv[:, :, k - 4]
                            nc.tensor.matmul(
                                ps[:, :],
                                w_sb[:, (k * 4 + m) * 128:(k * 4 + m + 1) * 128],
                                rhs, start=(k == 0), stop=(k == 7))
                        nc.scalar.activation(
                            out_v[:, :, m], ps[:, :], AF.Tanh,
                            bias=bhm_sb[:, bcol + m:bcol + m + 1])

                # local hs/ms rows over the window
                hs_row = p4pool.tile([1, wn], f32)
                ms_row = p4pool.tile([1, wn], f32)
                for (src_v, wcol, dst) in ((th_v, 0, hs_row), (tm_v, 4, ms_row)):
                    ps = p4ps1.tile([1, wn], f32, tag="vec")
                    for m in range(4):
                        nc.tensor.matmul(
                            ps[:, :], wout_sb[:, wcol + m:wcol + m + 1],
                            src_v[:, :, m], start=(m == 0), stop=(m == 3))
                    nc.vector.tensor_copy(dst[0:1, :], ps[:, :])

                if not go("full"):
                    raise _StopEmit()
                # valid-block rows -> [4, 128] lhsT (SBUF-SBUF DMA across partitions)
                lhsT4 = p4pool.tile([4, 128], f32)
                nc.sync.dma_start(out=lhsT4[0:1, :], in_=hs_row[0:1, w_warm:w_warm + 128])
                nc.sync.dma_start(out=lhsT4[1:2, :], in_=hs_row[0:1, w_warm + 128:w_warm + 256])
                nc.sync.dma_start(out=lhsT4[2:3, :], in_=ms_row[0:1, w_warm:w_warm + 128])
                nc.sync.dma_start(out=lhsT4[3:4, :], in_=ms_row[0:1, w_warm + 128:w_warm + 256])

                # placement matmul -> [128, 16] contribution; 8-party AllReduce
                contrib_ps = p4ps1.tile([128, 16], f32, tag="contrib")
                nc.tensor.matmul(contrib_ps[:, :], lhsT4[:, :], smat_sb[:, :],
                                 start=True, stop=True)
                contrib_sb = p4pool.tile([128, 16], f32)
                nc.vector.tensor_copy(contrib_sb[:, :], contrib_ps[:, :])
                nc.sync.dma_start(out=hm_in[:, :], in_=contrib_sb[:, :])
                if one_core:
                    nc.sync.dma_start(out=hm_out[:, :], in_=hm_in[:, :])
                else:
                    nc.gpsimd.collective_compute(
                        "AllReduce", mybir.AluOpType.add,
                        ins=[hm_in[:, :]], outs=[hm_out[:, :]],
                        replica_groups=all_group)
                hm_sb = p4pool.tile([128, 16], f32)
                nc.sync.dma_start(out=hm_sb[:, :], in_=hm_out[:, :])

                # hs columns -> global hs row [1, 1024]
                hs_g = p4pool.tile([1, L], f32)
                for j in range(8):
                    rps = p4ps1.tile([1, 128], f32, tag="row")
                    nc.tensor.matmul(rps[0:1, :], hm_sb[:, j:j + 1], id32[:, :],
                                     start=True, stop=True)
                    nc.vector.tensor_copy(hs_g[0:1, j * 128:(j + 1) * 128], rps[0:1, :])

                # HS[p, t] = hs_g[t] + b_out  (broadcast over partitions)
                ones128 = p4pool.tile([1, 128], f32)
                nc.vector.memset(ones128[:, :], 1.0)
                HS_sb = p4pool.tile([128, L], f32)
                for half in range(2):
                    hps = p4ps.tile([128, 512], f32, tag="hsb")
                    nc.tensor.matmul(hps[:, :], ones128[:, :],
                                     hs_g[0:1, half * 512:(half + 1) * 512],
                                     start=True, stop=True)
                    nc.vector.tensor_scalar_add(
                        HS_sb[:, half * 512:(half + 1) * 512], hps[:, :], float(b_out))

                # score rows chunk r: HS + msT[:, r]
                for r in range(8):
                    sc = p4sc.tile([128, L], f32, tag="sc")
                    nc.vector.tensor_scalar_add(sc[:, :], HS_sb[:, :], hm_sb[:, 8 + r:9 + r])
                    nc.sync.dma_start(out=score[r * 128:(r + 1) * 128, :], in_=sc[:, :])

    nc.compile()
    return nc


# ---------------------------------------------------------------- entry point

_CACHED = {}


def _get_nc(b_out: float):
    key = ("nc8", float(b_out))
    if key not in _CACHED:
        _CACHED[key] = build_nc(b_out=b_out)
    return _CACHED[key]


def kernel(**inputs) -> np.ndarray:
    from concourse.bass_utils import run_bass_kernel_spmd

    b_out = float(np.asarray(inputs["b_out"]).reshape(-1)[0])
    nc = _get_nc(b_out)
    in_maps = pack_inputs(inputs)
    res = run_bass_kernel_spmd(nc, in_maps, core_ids=list(range(8)))
    return np.concatenate(
        [np.asarray(res.results[k]["score"], dtype=np.float32) for k in range(8)],
        axis=0)
